# revision 1
# baseline (speedup 1.0000x reference)
"""Trainium2 Bass kernel for nn_Nequix (e3nn-style message-passing layer).

Sharding: edges partitioned by receiver range across 8 cores (500 nodes each);
node features and weights replicated; no collectives. Within a core, edges are
grouped into 4 windows of 128 receiver nodes; scatter-sum is one-hot matmuls
accumulating in PSUM per window.

Device layout summary (per core):
  y = [y0|y1x|y1y|y1z] : [4096, 512] bf16 in DRAM (replicated transform)
  m = y[senders]       : dma_gather -> [128, TW, 512] bf16 (edge t*128+p -> [p,t])
  radial MLP           : edges-on-free [64, E] tiles; L3 uses h3 as stationary
                         so w lands edge-on-partition [128e, 512] in PSUM
  messages             : [128e, 1024] bf16 = [s0|s1|v0x|v0y|v0z|v1x|v1y|v1z]
  scatter              : lhsT=one-hot(rloc) [128e, 128n]; agg PSUM [128n, 1024]
  final                : transpose agg, linear_2 + species-stacked skip matmuls,
                         silu gates; outT [512, 512] f32
"""
import math
import os
import numpy as np

KSTAGE = int(os.environ.get("KSTAGE", "9"))

N, E, C, NS, RB, H = 4000, 128000, 128, 5, 8, 64
AVG_N = 32.0
NCORES = 8
NV = 500
WIN = 128
NWIN = 4
NPAD = 4096


def _prep_host(inputs):
    import ml_dtypes
    bf = ml_dtypes.bfloat16
    f32 = np.float32

    xs = np.asarray(inputs["x_scalars"], f32)
    xv = np.asarray(inputs["x_vectors"], f32)
    ev = np.asarray(inputs["edge_vectors"], f32)
    rb = np.asarray(inputs["radial_basis"], f32)
    W1_0 = np.asarray(inputs["W1_0"], f32)
    W1_1 = np.asarray(inputs["W1_1"], f32)
    w0 = np.asarray(inputs["rmlp_w0"], f32)
    w1 = np.asarray(inputs["rmlp_w1"], f32)
    w2 = np.asarray(inputs["rmlp_w2"], f32)
    w3 = np.asarray(inputs["rmlp_w3"], f32).copy()
    W2_0 = np.asarray(inputs["W2_0"], f32)
    W2_1 = np.asarray(inputs["W2_1"], f32)
    Wsk0 = np.asarray(inputs["Wsk0"], f32)
    Wsk1 = np.asarray(inputs["Wsk1"], f32)
    species = np.asarray(inputs["species"]).astype(np.int64)
    send = np.asarray(inputs["senders"]).astype(np.int64)
    recv = np.asarray(inputs["receivers"]).astype(np.int64)

    inv_c = f32(1.0 / math.sqrt(C))
    W1_0f = W1_0 * inv_c
    W1_1f = W1_1 * inv_c
    w3f = w3 * f32(1.0 / math.sqrt(AVG_N))
    w3f[:, C:2 * C] *= f32(1.0 / math.sqrt(3.0))
    inv_2c = f32(1.0 / math.sqrt(2 * C))
    W2_0f = W2_0 * inv_2c
    W2_1f = W2_1 * inv_2c
    Wsk0f = Wsk0 * inv_c          # [NS, C, 2C]
    Wsk1f = Wsk1 * inv_c          # [NS, C, C]

    # ---- edge shard assignment
    core_of = recv // NV
    per_core_win = []
    maxcnt = 0
    for i in range(NCORES):
        eidx = np.nonzero(core_of == i)[0]
        r_loc = recv[eidx] - i * NV
        w_of = r_loc // WIN
        wins = []
        for w in range(NWIN):
            pw = eidx[w_of == w]
            order = np.argsort((recv[pw] - i * NV) % WIN, kind="stable")
            wins.append(pw[order])
            maxcnt = max(maxcnt, len(pw))
        per_core_win.append(wins)
    EW = ((maxcnt + 127) // 128) * 128
    EPAD = EW * NWIN
    TT = EPAD // 128

    # ---- shared constants (device layouts)
    xsT = np.zeros((C, NPAD), f32)
    xsT[:, :N] = xs.T
    xvT = np.zeros((3, C, NPAD), f32)
    for i in range(3):
        xvT[i, :, :N] = xv[:, :, i].T

    # W2 / Wsk stored [128 (c%128), kchunk, d]
    W20L = np.stack([W2_0f[0:128, :], W2_0f[128:256, :]], axis=1)      # [128,2,256]
    W21L = np.stack([W2_1f[0:128, :], W2_1f[128:256, :]], axis=1)      # [128,2,128]
    Wsk0L = Wsk0f.transpose(1, 0, 2)                                   # [128,NS,256]
    Wsk1L = Wsk1f.transpose(1, 0, 2)                                   # [128,NS,128]

    iota = np.tile(np.arange(WIN, dtype=f32)[None, :], (128, 1))

    consts = dict(
        xsT=xsT.astype(bf), xvT=xvT.astype(bf),
        W10=W1_0f.astype(bf), W11=W1_1f.astype(bf),
        w0=w0.astype(bf), w1=w1.astype(bf), w2=w2.astype(bf),
        w3=w3f.astype(bf),
        W20=W20L.astype(bf), W21=W21L.astype(bf),
        Wsk0=Wsk0L.astype(bf), Wsk1=Wsk1L.astype(bf),
        iota=iota.astype(bf),
    )

    # ---- per-core tensors
    cores = []
    for i in range(NCORES):
        send_p = np.zeros(EPAD, np.int64)
        rloc_p = np.zeros(EPAD, np.int64)
        ev_p = np.zeros((EPAD, 3), f32)
        ev_p[:, 0] = 1.0
        rb_p = np.zeros((EPAD, RB), f32)
        for w in range(NWIN):
            pw = per_core_win[i][w]
            k = len(pw)
            sl = slice(w * EW, w * EW + k)
            send_p[sl] = send[pw]
            rloc_p[sl] = (recv[pw] - i * NV) % WIN
            rb_p[sl] = rb[pw]
            ev_p[sl] = ev[pw]

        idx16 = send_p.astype(np.int16).reshape(EPAD // 16, 16).T
        sendidx = np.tile(idx16, (8, 1))                               # [128, EPAD//16]
        rlocT = rloc_p.reshape(TT, 128).T.astype(bf)                   # [128, TT]
        evT = ev_p.reshape(TT, 128, 3).transpose(1, 0, 2)              # [128, TT, 3]
        rbT = rb_p.T                                                   # [8, EPAD]

        sl_n = slice(i * NV, (i + 1) * NV)
        xs_my = np.zeros((512, C), f32)
        xs_my[:NV] = xs[sl_n]
        xv_my = np.zeros((512, C, 3), f32)
        xv_my[:NV] = xv[sl_n]
        soh = np.zeros((512, NS), f32)
        soh[np.arange(NV), species[sl_n]] = 1.0
        # xskT[c, k, n] = xs_my[n, c] * soh[n, k]
        xskT = np.einsum("nc,nk->ckn", xs_my, soh)                     # [128,NS,512]
        xvkT = np.einsum("nci,nk->ckin", xv_my, soh)                   # [128,NS,3,512]

        cores.append(dict(
            sendidx=np.ascontiguousarray(sendidx),
            rlocT=np.ascontiguousarray(rlocT),
            evT=np.ascontiguousarray(evT),
            rbT=np.ascontiguousarray(rbT.astype(bf)),
            xskT=np.ascontiguousarray(xskT.astype(bf)),
            xvkT=np.ascontiguousarray(xvkT.astype(bf)),
        ))
    return consts, cores, EW


def _build_program(EW):
    import concourse.bacc as bacc
    import concourse.mybir as mybir
    import concourse.tile as tile
    from concourse.masks import make_identity

    f32 = mybir.dt.float32
    bf = mybir.dt.bfloat16
    i16 = mybir.dt.int16
    AF = mybir.ActivationFunctionType
    OP = mybir.AluOpType

    EPAD = EW * NWIN
    TT = EPAD // 128
    TW = EW // 128

    nc = bacc.Bacc("TRN2", target_bir_lowering=False)

    def param(name, shape, dtype):
        return nc.declare_dram_parameter(name, list(shape), dtype, isOutput=False)

    xsT_d = param("xsT", (C, NPAD), bf)
    xvT_d = param("xvT", (3, C, NPAD), bf)
    W10_d = param("W10", (C, C), bf)
    W11_d = param("W11", (C, C), bf)
    w0_d = param("w0", (RB, H), bf)
    w1_d = param("w1", (H, H), bf)
    w2_d = param("w2", (H, H), bf)
    w3_d = param("w3", (H, 4 * C), bf)
    W20_d = param("W20", (C, 2, 2 * C), bf)
    W21_d = param("W21", (C, 2, C), bf)
    Wsk0_d = param("Wsk0", (C, NS, 2 * C), bf)
    Wsk1_d = param("Wsk1", (C, NS, C), bf)
    iota_d = param("iota", (128, WIN), bf)
    sendidx_d = param("sendidx", (128, EPAD // 16), i16)
    rlocT_d = param("rlocT", (128, TT), bf)
    evT_d = param("evT", (128, TT, 3), f32)
    rbT_d = param("rbT", (RB, EPAD), bf)
    xskT_d = param("xskT", (C, NS, 512), bf)
    xvkT_d = param("xvkT", (C, NS, 3, 512), bf)
    outT_d = nc.declare_dram_parameter("outT", [4 * C, 512], f32, isOutput=True)

    with tile.TileContext(nc) as tc:
        with (
            tc.tile_pool(name="dram", bufs=1, space="DRAM") as dpool,
            tc.tile_pool(name="const", bufs=1) as cpool,
            tc.tile_pool(name="xload", bufs=3) as xpool,
            tc.tile_pool(name="ybuf", bufs=3) as ypool,
            tc.tile_pool(name="gather", bufs=2) as gpool,
            tc.tile_pool(name="edge", bufs=2) as epool,
            tc.tile_pool(name="mlp", bufs=2) as hpool,
            tc.tile_pool(name="msg", bufs=4) as mpool,
            tc.tile_pool(name="fin", bufs=2) as fpool,
            tc.tile_pool(name="ps_small", bufs=2, space="PSUM") as ps_small,
            tc.tile_pool(name="ps_w", bufs=2, space="PSUM") as ps_w,
            tc.tile_pool(name="ps_agg", bufs=1, space="PSUM") as ps_agg,
        ):
            y_d = dpool.tile([NPAD, 4 * C], bf)

            def cload(dram, shape, dtype):
                t = cpool.tile(list(shape), dtype, tag=dram.name)
                nc.sync.dma_start(t[:], dram[:])
                return t

            W10_s = cload(W10_d, (C, C), bf)
            W11_s = cload(W11_d, (C, C), bf)
            w0_s = cload(w0_d, (RB, H), bf)
            w1_s = cload(w1_d, (H, H), bf)
            w2_s = cload(w2_d, (H, H), bf)
            w3_s = cload(w3_d, (H, 4 * C), bf)
            iota_s = cload(iota_d, (128, WIN), bf)
            sendidx_s = cload(sendidx_d, (128, EPAD // 16), i16)
            rlocT_s = cload(rlocT_d, (128, TT), bf)
            W20_s = cload(W20_d, (C, 2, 2 * C), bf)
            W21_s = cload(W21_d, (C, 2, C), bf)
            Wsk0_s = cload(Wsk0_d, (C, NS, 2 * C), bf)
            Wsk1_s = cload(Wsk1_d, (C, NS, C), bf)
            xskT_s = cload(xskT_d, (C, NS, 512), bf)
            xvkT_s = cload(xvkT_d, (C, NS, 3, 512), bf)
            ident_s = cpool.tile([128, 128], bf)
            make_identity(nc, ident_s[:])

            # ================= phase Y =================
            for nch in range(NPAD // 128):
                xs_t = xpool.tile([C, 128], bf, tag="xs")
                nc.sync.dma_start(xs_t[:], xsT_d[:, nch * 128:(nch + 1) * 128])
                psy = ps_w.tile([128, 8 * C], f32, tag="w")
                nc.tensor.matmul(psy[:, 0:C], lhsT=xs_t[:], rhs=W10_s[:],
                                 start=True, stop=True)
                for i in range(3):
                    xv_t = xpool.tile([C, 128], bf, tag="xv")
                    nc.sync.dma_start(xv_t[:], xvT_d[i, :, nch * 128:(nch + 1) * 128])
                    nc.tensor.matmul(psy[:, (1 + i) * C:(2 + i) * C], lhsT=xv_t[:],
                                     rhs=W11_s[:], start=True, stop=True)
                yb = ypool.tile([128, 4 * C], bf)
                nc.vector.tensor_copy(yb[:], psy[:, 0:4 * C])
                nc.sync.dma_start(y_d[nch * 128:(nch + 1) * 128, :], yb[:])

            # ================= per window =================
            for w in range(NWIN if KSTAGE >= 2 else 0):
                m_sb = gpool.tile([128, TW, 4 * C], bf)
                # SWDGE descriptor ring limits one dma_gather to ~1k indices
                for c0 in range(0, TW, 8):
                    c1 = min(c0 + 8, TW)
                    nidx = (c1 - c0) * 128
                    i0 = (w * EW + c0 * 128) // 16
                    nc.gpsimd.dma_gather(
                        m_sb[:, c0:c1, :], y_d[:],
                        sendidx_s[:, i0:i0 + nidx // 16],
                        nidx, nidx, 4 * C,
                    )
                ev_sb = epool.tile([128, TW, 3], f32, tag="ev")
                nc.sync.dma_start(ev_sb[:], evT_d[:, w * TW:(w + 1) * TW, :])
                rbT_sb = epool.tile([RB, EW], bf, tag="rb")
                nc.sync.dma_start(rbT_sb[:], rbT_d[:, w * EW:(w + 1) * EW])

                # ---- edge geometry -> Y1 components [128, 3, TW] f32
                sq = epool.tile([128, TW, 3], f32, tag="sq")
                nc.vector.tensor_tensor(out=sq[:], in0=ev_sb[:], in1=ev_sb[:],
                                        op=OP.mult)
                r2 = epool.tile([128, TW], f32, tag="r2")
                nc.vector.tensor_tensor(out=r2[:], in0=sq[:, :, 0], in1=sq[:, :, 1],
                                        op=OP.add)
                nc.vector.tensor_tensor(out=r2[:], in0=r2[:], in1=sq[:, :, 2],
                                        op=OP.add)
                rr = epool.tile([128, TW], f32, tag="rr")
                nc.scalar.sqrt(rr[:], r2[:])
                nc.vector.tensor_scalar_max(rr[:], rr[:], 1e-12)
                rinv = epool.tile([128, TW], f32, tag="rinv")
                nc.vector.reciprocal(rinv[:], rr[:])
                Y1 = epool.tile([128, 3, TW], f32, tag="Y1")
                for i in range(3):
                    nc.vector.scalar_tensor_tensor(
                        out=Y1[:, i, :], in0=ev_sb[:, :, i],
                        scalar=float(math.sqrt(3.0)), in1=rinv[:],
                        op0=OP.mult, op1=OP.mult)

                agg = ps_agg.tile([128, 8 * C], f32)

                pairs = [(p0, min(p0 + 2, TW)) for p0 in range(0, TW, 2)] if KSTAGE >= 3 else []
                for (t0, t1) in pairs:
                    npair = t1 - t0
                    ne = npair * 128
                    e0 = t0 * 128
                    # ---- radial MLP (edges on free dim)
                    psh1 = ps_small.tile([64, 256], f32, tag="sm")
                    nc.tensor.matmul(psh1[0:H, :ne], lhsT=w0_s[:],
                                     rhs=rbT_sb[:, e0:e0 + ne], start=True, stop=True)
                    h1 = hpool.tile([64, 256], bf, tag="h1")
                    nc.scalar.activation(h1[:, :ne], psh1[0:H, :ne], AF.Silu)
                    psh2 = ps_small.tile([64, 256], f32, tag="sm")
                    nc.tensor.matmul(psh2[0:H, :ne], lhsT=w1_s[:], rhs=h1[:, :ne],
                                     start=True, stop=True)
                    h2 = hpool.tile([64, 256], bf, tag="h2")
                    nc.scalar.activation(h2[:, :ne], psh2[0:H, :ne], AF.Silu)
                    psh3 = ps_small.tile([64, 256], f32, tag="sm")
                    nc.tensor.matmul(psh3[0:H, :ne], lhsT=w2_s[:], rhs=h2[:, :ne],
                                     start=True, stop=True)
                    h3 = hpool.tile([64, 256], bf, tag="h3")
                    nc.scalar.activation(h3[:, :ne], psh3[0:H, :ne], AF.Silu)

                    # ---- L3: w[e, 4C], edge-on-partition, PSUM
                    psw = ps_w.tile([128, 2, 4 * C], f32, tag="w")
                    for tt_ in range(npair):
                        nc.tensor.matmul(psw[:, tt_, :],
                                         lhsT=h3[:, tt_ * 128:(tt_ + 1) * 128],
                                         rhs=w3_s[:], start=True, stop=True)
                    # evacuate w -> bf16 SBUF (DVE half / ACT half)
                    w_sb = mpool.tile([128, 2, 4 * C], bf, tag="wsb")
                    nc.vector.tensor_copy(w_sb[:, 0:npair, 0:2 * C],
                                          psw[:, 0:npair, 0:2 * C])
                    nc.scalar.copy(w_sb[:, 0:npair, 2 * C:4 * C],
                                   psw[:, 0:npair, 2 * C:4 * C])

                    # ---- messages [128, npair, 8C] bf16
                    msg = mpool.tile([128, 2, 8 * C], bf, tag="msg")
                    mp = m_sb[:, t0:t1, :]
                    wp = w_sb[:, 0:npair, :]
                    nc.vector.tensor_tensor(out=msg[:, 0:npair, 0:C],
                                            in0=mp[:, :, 0:C], in1=wp[:, :, 0:C],
                                            op=OP.mult)
                    dot = mpool.tile([128, 2, C], bf, tag="dot")
                    for tt_ in range(npair):
                        t = t0 + tt_
                        nc.vector.tensor_scalar(
                            out=dot[:, tt_, :], in0=m_sb[:, t, C:2 * C],
                            scalar1=Y1[:, 0, t:t + 1], scalar2=None, op0=OP.mult)
                        for i in (1, 2):
                            nc.vector.scalar_tensor_tensor(
                                out=dot[:, tt_, :],
                                in0=m_sb[:, t, (1 + i) * C:(2 + i) * C],
                                scalar=Y1[:, i, t:t + 1], in1=dot[:, tt_, :],
                                op0=OP.mult, op1=OP.add)
                    nc.vector.tensor_tensor(out=msg[:, 0:npair, C:2 * C],
                                            in0=dot[:, 0:npair, :],
                                            in1=wp[:, :, C:2 * C], op=OP.mult)
                    wv0m = mpool.tile([128, 2, C], bf, tag="wv0m")
                    nc.vector.tensor_tensor(out=wv0m[:, 0:npair, :],
                                            in0=mp[:, :, 0:C],
                                            in1=wp[:, :, 2 * C:3 * C], op=OP.mult)
                    for tt_ in range(npair):
                        t = t0 + tt_
                        for i in range(3):
                            nc.vector.tensor_scalar(
                                out=msg[:, tt_, (2 + i) * C:(3 + i) * C],
                                in0=wv0m[:, tt_, :], scalar1=Y1[:, i, t:t + 1],
                                scalar2=None, op0=OP.mult)
                    for i in range(3):
                        nc.vector.tensor_tensor(
                            out=msg[:, 0:npair, (5 + i) * C:(6 + i) * C],
                            in0=mp[:, :, (1 + i) * C:(2 + i) * C],
                            in1=wp[:, :, 3 * C:4 * C], op=OP.mult)

                    # ---- scatter
                    for tt_ in range(npair):
                        t = t0 + tt_
                        oh = mpool.tile([128, WIN], bf, tag="oh")
                        nc.vector.tensor_tensor(
                            out=oh[:],
                            in0=rlocT_s[:, w * TW + t:w * TW + t + 1]
                                .to_broadcast([128, WIN]),
                            in1=iota_s[:], op=OP.is_equal)
                        first = (t == 0)
                        last = (t == TW - 1)
                        nc.tensor.matmul(agg[:, 0:512], lhsT=oh[:],
                                         rhs=msg[:, tt_, 0:512],
                                         start=first, stop=last,
                                         skip_group_check=True)
                        nc.tensor.matmul(agg[:, 512:1024], lhsT=oh[:],
                                         rhs=msg[:, tt_, 512:1024],
                                         start=first, stop=last,
                                         skip_group_check=True)

                # ================= final per window =================
                if KSTAGE < 4:
                    continue
                agg_sb = fpool.tile([128, 8 * C], bf, tag="aggsb")
                nc.vector.tensor_copy(agg_sb[:, 0:512], agg[:, 0:512])
                nc.scalar.copy(agg_sb[:, 512:1024], agg[:, 512:1024])

                aggT = fpool.tile([128, 8, 128], bf, tag="aggT")
                for fch in range(8):
                    pst = ps_small.tile([128, 128], bf, tag="sm")
                    nc.tensor.transpose(pst[:], agg_sb[:, fch * 128:(fch + 1) * 128],
                                        identity=ident_s[:])
                    nc.vector.tensor_copy(aggT[:, fch, :], pst[:])

                # sT [2 chunks of 128 feat, 128 nodes]
                pss = ps_small.tile([128, 2, 128], f32, tag="sm")
                for mch in range(2):
                    for kch in range(2):
                        nc.tensor.matmul(
                            pss[:, mch, :],
                            lhsT=W20_s[:, kch, mch * 128:(mch + 1) * 128],
                            rhs=aggT[:, kch, :], start=(kch == 0), stop=False,
                            skip_group_check=True)
                    for k in range(NS):
                        nc.tensor.matmul(
                            pss[:, mch, :],
                            lhsT=Wsk0_s[:, k, mch * 128:(mch + 1) * 128],
                            rhs=xskT_s[:, k, w * 128:(w + 1) * 128],
                            start=False, stop=(k == NS - 1), skip_group_check=True)
                outs = fpool.tile([128, 128], bf, tag="outs")
                nc.scalar.activation(outs[:], pss[:, 0, :], AF.Silu)
                gates = fpool.tile([128, 128], bf, tag="gates")
                nc.scalar.activation(gates[:], pss[:, 1, :], AF.Silu)

                psv = ps_small.tile([128, 3, 128], f32, tag="sm")
                for i in range(3):
                    for kch in range(2):
                        nc.tensor.matmul(
                            psv[:, i, :], lhsT=W21_s[:, kch, :],
                            rhs=aggT[:, (2 + i) if kch == 0 else (5 + i), :],
                            start=(kch == 0), stop=False, skip_group_check=True)
                    for k in range(NS):
                        nc.tensor.matmul(
                            psv[:, i, :], lhsT=Wsk1_s[:, k, :],
                            rhs=xvkT_s[:, k, i, w * 128:(w + 1) * 128],
                            start=False, stop=(k == NS - 1), skip_group_check=True)

                ow = fpool.tile([128, 4, 128], f32, tag="ow")
                nc.vector.tensor_copy(ow[:, 0, :], outs[:])
                for i in range(3):
                    nc.vector.tensor_tensor(out=ow[:, 1 + i, :], in0=psv[:, i, :],
                                            in1=gates[:], op=OP.mult)
                for fch in range(4):
                    nc.sync.dma_start(
                        outT_d[fch * 128:(fch + 1) * 128, w * 128:(w + 1) * 128],
                        ow[:, fch, :])
    nc.compile()
    return nc


_CACHE = {}


def kernel(**inputs):
    from concourse.bass_utils import run_bass_kernel_spmd
    consts, cores, EW = _prep_host(inputs)
    if EW not in _CACHE:
        _CACHE[EW] = _build_program(EW)
    nc = _CACHE[EW]
    in_maps = []
    for i in range(NCORES):
        m = dict(consts)
        m.update(cores[i])
        in_maps.append(m)
    res = run_bass_kernel_spmd(nc, in_maps, list(range(NCORES)))
    out = np.zeros((NCORES, NV, 4 * C), np.float32)
    for i in range(NCORES):
        outT = np.asarray(res.results[i]["outT"], np.float32)   # [512, 512]
        full = outT[:, :NV].T                                   # [NV, 512]
        out_s = full[:, 0:C]
        v = np.stack([full[:, C:2 * C], full[:, 2 * C:3 * C], full[:, 3 * C:]],
                     axis=2).reshape(NV, 3 * C)
        out[i] = np.concatenate([out_s, v], axis=1)
    return out.reshape(N, 4 * C).astype(np.float32)

